# revision 1
# baseline (speedup 1.0000x reference)
"""EquiAttention Trainium2 kernel.

Computes the reference nn_EquiAttention forward pass on 8 NeuronCores,
data-parallel over the batch axis (64 batches -> 8 per core).

Math refactoring (validated exact in float64):
  The reference builds q/k embeddings of width 192:
    q = [ (Wq @ vecs).flat (128) , scalars @ Wq_s.T + bq_s (64) ]
    k = [ (Wk @ vecs * METRIC).flat (128) , scalars @ Wk_s.T + bk_s (64) ]
  Softmax over keys is invariant to per-query constants, so scores fold
  to a 128-dim contraction plus a per-key bias:
    scores[q,m] ~ qv_q.kv_m + s_q.(H s_m) + c2.s_m    (mod per-q const)
  with  qv = vecs.flat (64),  kv[(j,k),m] = scale*METRIC[k]*(G @ vecs[..,k])
        G = Wq.T @ Wk,  H = scale * Wq_s.T @ Wk_s,  c2 = scale * Wk_s.T @ bq_s
  The per-key bias folds into V:  w_m = exp(c2.s_m) (values ~[0.6,1.6]),
  Vaug[m] = [w_m * v_m, w_m];  out = acc[:, :64] / acc[:, 64].

Device structure per batch (per core):
  - qT [128,N] = [vecsT ; scalarsT] via PE transposes of the combined
    normalized-vector/scalar chunks; kT = blockdiag(G~,H~).T @ qT.
    Both are split hi/lo into fp16 pairs; 3-pass scores
    (qhi.khi + qhi.klo + qlo.khi) are exact to ~1e-4 absolute because
    max_row sum|q||k| ~ 117 (no catastrophic cancellation), so the PE
    runs at 1 cycle/row instead of fp32's 4.
  - scores per 128-query block land in two half-bank PSUM tiles
    (4-deep pool -> deep matmul/softmax pipelining); row-max via DVE
    reduce per half + min; P = exp(S-max) written fp16 by ACT.
  - P^T via DMA xbar transpose, two query blocks per DMA so each
    partition writes a 4KB contiguous run (avoids the M2S-concat
    bandwidth penalty); all xbar transposes on one HWDGE queue --
    concurrent xbar use from two queues returned corrupt data on HW.
  - P @ Vaug computed transposed per query-half: accT[65, 512] +=
    Vaug[mc].T @ P^T chunks (fp16, 512-wide moving), PE-transposed back
    per query block, normalized by the denominator column, and written
    out with one DMA per half.
"""

import numpy as np

B, N = 64, 1024
NCORES = 8
BL = B // NCORES          # batches per core
NB = N // 128             # 128-row blocks per sequence
SCALE = 1.0 / np.sqrt(192.0)

_CACHE = {}


def _build_program():
    import concourse.bacc as bacc
    import concourse.tile as tile
    from concourse import mybir

    f32 = mybir.dt.float32

    nc = bacc.Bacc("TRN2", target_bir_lowering=False,
                   debug=False, num_devices=NCORES)

    aps = {
        "vectors": nc.dram_tensor("vectors", [BL, N, 64], f32,
                                  kind="ExternalInput").ap(),
        "scalars": nc.dram_tensor("scalars", [BL, N, 64], f32,
                                  kind="ExternalInput").ap(),
        "BD": nc.dram_tensor("BD", [128, 128], f32, kind="ExternalInput").ap(),
        "WvC2": nc.dram_tensor("WvC2", [128, 65], f32, kind="ExternalInput").ap(),
        "out": nc.dram_tensor("out", [BL, N, 64], f32, kind="ExternalOutput").ap(),
    }

    with tile.TileContext(nc) as tc:
        _emit(tc, aps)

    nc.compile()
    return nc


def _emit(tc, aps):
    from contextlib import ExitStack
    import concourse.bass as bass
    import concourse.masks as masks
    from concourse import mybir

    nc = tc.nc
    f32 = mybir.dt.float32
    f16 = mybir.dt.float16
    PS = "PSUM"
    Act = mybir.ActivationFunctionType
    Alu = mybir.AluOpType
    X = mybir.AxisListType.X

    vecs_d, scal_d = aps["vectors"], aps["scalars"]
    bd_d, wvc2_d, out_d = aps["BD"], aps["WvC2"], aps["out"]

    with ExitStack() as ctx:
        singles = ctx.enter_context(tc.tile_pool(name="singles", bufs=1))
        raw = ctx.enter_context(tc.tile_pool(name="raw", bufs=2))
        emb = ctx.enter_context(tc.tile_pool(name="emb", bufs=2))
        small = ctx.enter_context(tc.tile_pool(name="small", bufs=6))
        pP = ctx.enter_context(tc.tile_pool(name="pP", bufs=3))
        pPT = ctx.enter_context(tc.tile_pool(name="pPT", bufs=2))
        outp = ctx.enter_context(tc.tile_pool(name="outp", bufs=4))
        accsb = ctx.enter_context(tc.tile_pool(name="accsb", bufs=4))
        psS = ctx.enter_context(tc.tile_pool(name="psS", bufs=5, space=PS))
        psAcc = ctx.enter_context(tc.tile_pool(name="psAcc", bufs=1, space=PS))
        psMisc = ctx.enter_context(tc.tile_pool(name="psMisc", bufs=2, space=PS))

        ident = singles.tile([128, 128], f32)
        masks.make_identity(nc, ident[:])
        bd = singles.tile([128, 128], f32)
        nc.gpsimd.dma_start(out=bd[:], in_=bd_d[:, :])
        bdhi = singles.tile([128, 128], f16)
        nc.vector.tensor_copy(bdhi[:], bd[:])
        bdlo = singles.tile([128, 128], f16)
        nc.vector.tensor_sub(bdlo[:], bd[:], bdhi[:])
        wvc2_16 = singles.tile([128, 65], f16)
        nc.gpsimd.dma_start(out=wvc2_16[:], in_=wvc2_d[:, :].bitcast(f32))

        def embed_pre(b):
            # ---------- embedding: DMA + normalize (no PE work) ----------
            # combined [vec | scalar] chunk tile so one PE transpose per
            # chunk yields a full 128-row column block of qT
            vs = raw.tile([128, NB, 128], f32, tag="vs")
            nc.gpsimd.dma_start(out=vs[:, :, 64:128],
                                in_=scal_d[b].rearrange("(c p) f -> p c f", p=128))
            vraw = raw.tile([128, NB, 64], f32, tag="vraw")
            nc.gpsimd.dma_start(out=vraw[:],
                                in_=vecs_d[b].rearrange("(c p) f -> p c f", p=128))

            # Lorentz normalization of the 16 four-vectors per particle
            sq = raw.tile([128, NB, 16, 4], f32, tag="sq")
            nc.scalar.activation(out=sq[:], in_=vraw[:], func=Act.Square)
            nrm = raw.tile([128, NB, 16], f32, tag="nrm")
            nc.vector.tensor_sub(nrm[:], sq[:, :, :, 0], sq[:, :, :, 1])
            nc.vector.tensor_sub(nrm[:], nrm[:], sq[:, :, :, 2])
            nc.vector.tensor_sub(nrm[:], nrm[:], sq[:, :, :, 3])
            nc.scalar.activation(out=nrm[:], in_=nrm[:], func=Act.Abs)
            nc.vector.tensor_scalar_max(nrm[:], nrm[:], 1e-5)
            nc.scalar.activation(out=nrm[:], in_=nrm[:], func=Act.Sqrt)
            rn = raw.tile([128, NB, 16], f32, tag="rn")
            nc.vector.reciprocal(rn[:], nrm[:])
            return vs, vraw, rn

        def embed_pe(vs, vraw, rn):
            # ---------- embedding: PE transposes + projections ----------
            qT = emb.tile([128, N], f32, tag="qT")
            qhi = emb.tile([128, N], f16, tag="qhi")
            qlo = emb.tile([128, N], f16, tag="qlo")
            khi = emb.tile([128, N], f16, tag="khi")
            klo = emb.tile([128, N], f16, tag="klo")
            half = NB // 2
            for hh in range(2):
                cs = slice(hh * half, (hh + 1) * half)
                rn_b = bass.AP(tensor=rn.tensor,
                               offset=rn.offset + hh * half * rn.ap[1][0],
                               ap=[rn.ap[0], [rn.ap[1][0], half], rn.ap[2],
                                   [0, 4]])
                nc.vector.tensor_mul(
                    vs[:, cs, 0:64].rearrange("p c (j k) -> p c j k", k=4),
                    vraw[:, cs].rearrange("p c (j k) -> p c j k", k=4), rn_b)
                # four transposes into one PSUM bank, then one copy
                pt = psMisc.tile([128, 512], f32, tag="misc")
                for j, c in enumerate(range(hh * half, (hh + 1) * half)):
                    nc.tensor.transpose(pt[:, j * 128:(j + 1) * 128],
                                        vs[:, c], ident[:])
                nc.vector.tensor_copy(qT[:, hh * 512:(hh + 1) * 512], pt[:])
                # fp16 hi/lo split of qT; 3-pass scores
                # qhi.khi + qhi.klo + qlo.khi are exact to ~1e-4 (max_row
                # sum|q||k| ~ 117 -> no catastrophic cancellation)
                cols = slice(hh * 512, (hh + 1) * 512)
                nc.vector.tensor_copy(qhi[:, cols], qT[:, cols])
                nc.vector.tensor_sub(qlo[:, cols], qT[:, cols], qhi[:, cols])
                # kT = blockdiag(G~, H~).T @ qT, hi/lo split from PSUM
                pk = psMisc.tile([128, 512], f32, tag="misc")
                nc.tensor.matmul(pk[:], bdhi[:], qhi[:, cols],
                                 start=True, stop=False)
                nc.tensor.matmul(pk[:], bdhi[:], qlo[:, cols],
                                 start=False, stop=False)
                nc.tensor.matmul(pk[:], bdlo[:], qhi[:, cols],
                                 start=False, stop=True)
                nc.scalar.copy(khi[:, cols], pk[:])
                nc.vector.tensor_sub(klo[:, cols], pk[:], khi[:, cols])

            # Vaug chunks (natural key order, matching the xbar block
            # transpose): Vaug[m] = [w_m * v_m, w_m], w = exp(c2.s)
            vaug = emb.tile([128, NB, 65], f16, tag="vaug")
            for mc in range(NB):
                csel = qhi[:, mc * 128:(mc + 1) * 128]
                pv = psMisc.tile([128, 65], f32, tag="misc")
                nc.tensor.matmul(pv[:], csel, wvc2_16[:], start=True, stop=True)
                nc.scalar.activation(out=vaug[:, mc, 64:65], in_=pv[:, 64:65],
                                     func=Act.Exp)
                wcol = small.tile([128, 1], f32, tag="wcol")
                nc.scalar.activation(out=wcol[:], in_=pv[:, 64:65], func=Act.Exp)
                nc.scalar.activation(out=vaug[:, mc, 0:64], in_=pv[:, 0:64],
                                     func=Act.Copy, scale=wcol[:])
            return qhi, qlo, khi, klo, vaug

        def attn_qblocks(emb_tiles):
            qhi, qlo, khi, klo, vaug = emb_tiles
            # ---------------- attention phase ----------------
            # P^T layout: ptf[p, qb, mc, q'] = P[qb*128+q', mc*128+p];
            # dims ordered so each DMA-transpose destination ptf[:, qb]
            # is contiguous per partition (sliced dst is wrong on HW)
            ptf = pPT.tile([128, NB, NB, 128], f16, tag="ptf")

            def q_block(qb, P2):
                qs = slice(qb * 128, (qb + 1) * 128)
                Sh, m01 = [], []
                for h in range(2):
                    cols = slice(h * 512, (h + 1) * 512)
                    S = psS.tile([128, 512], f32, tag="S")
                    nc.tensor.matmul(S[:], qhi[:, qs], khi[:, cols],
                                     start=True, stop=False)
                    nc.tensor.matmul(S[:], qhi[:, qs], klo[:, cols],
                                     start=False, stop=False)
                    nc.tensor.matmul(S[:], qlo[:, qs], khi[:, cols],
                                     start=False, stop=True)
                    m = small.tile([128, 1], f32, tag="m01")
                    nc.vector.tensor_reduce(m[:], S[:], axis=X,
                                            op=Alu.max, negate=True)
                    Sh.append(S)
                    m01.append(m)
                negmax = small.tile([128, 1], f32, tag="negmax")
                nc.vector.tensor_tensor(negmax[:], m01[0][:], m01[1][:],
                                        op=Alu.min)
                if P2 is None:
                    P2 = pP.tile([128, 2, N], f16, tag="P")
                for h in range(2):
                    nc.scalar.activation(
                        out=P2[:, qb % 2, h * 512:(h + 1) * 512],
                        in_=Sh[h][:], func=Act.Exp,
                        bias=negmax[:], scale=1.0)
                if qb % 2 == 1:
                    # two query blocks per xbar transpose: 4KB contiguous
                    # runs per partition (vs 2KB) avoid the M2S-concat
                    # bandwidth penalty and halve the DMA count
                    nc.sync.dma_start_transpose(
                        ptf[:, qb - 1:qb + 1],
                        P2[:].rearrange("p two m -> p (two m)"))
                return P2

            # accT[65, qhalf] += Vaug[mc].T @ P^T[mc] (fp16, 512-wide);
            # per-half acc banks so the epilogue starts mid-batch
            P2 = None
            for qb in range(NB):
                P2 = q_block(qb, P2)
                if qb % 2 == 1:
                    P2 = None
            return ptf

        def attn_pv_epi(b, emb_tiles, ptf):
            qhi, qlo, khi, klo, vaug = emb_tiles

            def pv_epi(hh):
                accT = psAcc.tile([65, 512], f32, tag="accT")
                for mc in range(NB):
                    nc.tensor.matmul(accT[:], vaug[:, mc, :],
                                     ptf[:, hh * 4:(hh + 1) * 4, mc, :],
                                     start=(mc == 0), stop=(mc == NB - 1))
                accsb_t = accsb.tile([65, 512], f32, tag="accsb")
                nc.vector.tensor_copy(accsb_t[:], accT[:])
                ot = psMisc.tile([128, 4, 65], f32, tag="misc")
                for j in range(4):
                    nc.tensor.transpose(ot[:, j], accsb_t[:, j * 128:(j + 1) * 128],
                                        ident[0:65, 0:65])
                rden = small.tile([128, 4], f32, tag="rden")
                nc.vector.reciprocal(rden[:], ot[:, :, 64])
                ob = outp.tile([128, 4, 64], f32, tag="ob")
                for j in range(4):
                    nc.vector.tensor_scalar_mul(ob[:, j], ot[:, j, 0:64],
                                                rden[:, j:j + 1])
                nc.gpsimd.dma_start(
                    out=out_d[b, hh * 512:(hh + 1) * 512, :]
                    .rearrange("(j p) f -> p j f", p=128),
                    in_=ob[:])

            pv_epi(0)
            pv_epi(1)

        # One-batch-ahead software pipelining: embed(b+1) is emitted
        # (and thus prioritized) before attention(b).
        prev = embed_pe(*embed_pre(0))
        for b in range(1, BL):
            cur = embed_pe(*embed_pre(b))
            attn_pv_epi(b - 1, prev, attn_qblocks(prev))
            prev = cur
        attn_pv_epi(BL - 1, prev, attn_qblocks(prev))


def _host_weights(Wq, Wk, Wv, Wq_s, Wk_s, bq_s):
    """Fold the tiny EquiLinear weights (float64 precompute, cast f32)."""
    METRIC = np.array([1.0, -1.0, -1.0, -1.0], dtype=np.float64)
    G = Wq.astype(np.float64).T @ Wk.astype(np.float64)            # [16,16]
    BD = np.zeros((128, 128), dtype=np.float64)
    for k in range(4):
        # lhsT[(j',k), (j,k)] = SCALE * METRIC[k] * G[j, j']
        BD[k:64:4, k:64:4] = SCALE * METRIC[k] * G.T
    # lhsT[h, g] = SCALE * H[g, h],  H = Wq_s.T @ Wk_s
    BD[64:, 64:] = SCALE * (Wk_s.astype(np.float64).T @ Wq_s.astype(np.float64))
    E = np.exp(Wv.astype(np.float64))                              # [16,16]
    WvC2 = np.zeros((128, 65), dtype=np.float64)
    for k in range(4):
        # rhs[(j,k), (i,k)] = E[i, j]
        WvC2[k:64:4, k:64:4] = E.T
    WvC2[64:, 64] = SCALE * (Wk_s.astype(np.float64).T @ bq_s.astype(np.float64))
    return (np.ascontiguousarray(BD, dtype=np.float32),
            np.ascontiguousarray(WvC2, dtype=np.float32))


def _prepare_in_maps(vectors, scalars, Wq, Wq_s, bq_s, Wk, Wk_s, bk_s, Wv):
    BD, WvC2 = _host_weights(Wq, Wk, Wv, Wq_s, Wk_s, bq_s)
    vecs_flat = np.ascontiguousarray(
        np.asarray(vectors).reshape(B, N, 64), dtype=np.float32)
    scal = np.ascontiguousarray(scalars, dtype=np.float32)

    in_maps = []
    for c in range(NCORES):
        sl = slice(c * BL, (c + 1) * BL)
        in_maps.append({
            "vectors": np.ascontiguousarray(vecs_flat[sl]),
            "scalars": np.ascontiguousarray(scal[sl]),
            "BD": BD,
            "WvC2": WvC2,
        })
    return in_maps


def _run(in_maps, **kw):
    from concourse.bass_utils import run_bass_kernel_spmd
    nc = _get_program()
    return run_bass_kernel_spmd(nc, in_maps, list(range(NCORES)), **kw)


def _get_program():
    if "nc" not in _CACHE:
        _CACHE["nc"] = _build_program()
    return _CACHE["nc"]


def kernel(vectors, scalars, Wq, Wq_s, bq_s, Wk, Wk_s, bk_s, Wv):
    args = [np.asarray(a, dtype=np.float32) for a in
            (vectors, scalars, Wq, Wq_s, bq_s, Wk, Wk_s, bk_s, Wv)]
    in_maps = _prepare_in_maps(*args)
    res = _run(in_maps)
    out = np.concatenate([res.results[c]["out"] for c in range(NCORES)], axis=0)
    return out.reshape(B, N, 16, 4).astype(np.float32)



# revision 3
# speedup vs baseline: 2.2321x; 2.2321x over previous
"""EquiAttention Trainium2 kernel (v2).

Data-parallel over batch: 64 batches -> 8 per core, seq N=1024.

Math (folded form validated exact in float64, see _host_weights):
  softmax scores fold to a 128-dim contraction plus a per-key bias:
    S[q,m] ~ q128_q . (BD^T q128)_m + c2.s_m   (mod per-query const)
  with q128 = [normalized vecs (64) | scalars (64)].
  V = (exp(Wv)-proj vecs); denominator via an all-ones 65th V column.

Device structure per batch (all layouts feature-major, prepped on host):
  - qT [128,N] f32r arrives pre-normalized/transposed from host (the
    O(B*N) pointwise Lorentz normalization + layout transform are host
    prep; all O(B*N^2) attention math runs on device).
  - kT = BD^T qT: one fp32r matmul pair (fp32r streams 1 cycle/row at
    moving>=256 vs fp32's 4 -- measured 230ns vs 860ns per 512-wide).
  - Scores are computed TRANSPOSED, per key-chunk mc:
      S^T[mc] [128m, 1024q] = matmul(kt chunk stationary, qT moving)
    so exp(S^T) IS P^T directly -- no DMA/PE transposes of P at all.
  - exp: one 1024-wide ACT instruction per chunk reading the 2-bank
    PSUM tile, per-partition bias = c2.s_m - 20 (constant shift instead
    of a per-row max: removes the DVE row-max reduce and its serial
    dependency). P stored bf16 (dynamic range to 3e38).
  - PV: accT[65,1024] += Vaug[mc]^T @ P^T[mc] (bf16), denominator from
    the ones column.  accT is DMA'd out raw as [65,N]; the final
    divide + [65,N]->[N,64] transpose happen on host (0.05% of FLOPs).
  - Outlier rows (rowmax > ~85 after the -20 shift overflow fp32/bf16;
    18 of 65536 rows in this distribution, winner-take-all structure)
    are detected on host via den >= e^65 or nonfinite and recomputed
    exactly in numpy (~0.3 MFLOP/row).
"""

import numpy as np

B, N = 64, 1024
NCORES = 8
BL = B // NCORES          # batches per core
NB = N // 128             # 128-row key chunks
SCALE = 1.0 / np.sqrt(192.0)
CSHIFT = 20.0             # constant softmax shift (rowmax p99.9 = 48)
FLAG_LOGDEN = 65.0        # host-recompute rows with log(den) above this

_CACHE = {}


def _build_program():
    import concourse.bacc as bacc
    import concourse.tile as tile
    from concourse import mybir

    f32 = mybir.dt.float32
    f32r = mybir.dt.float32r
    f16 = mybir.dt.float16

    nc = bacc.Bacc("TRN2", target_bir_lowering=False,
                   debug=False, num_devices=NCORES)

    aps = {
        "qT": nc.dram_tensor("qT", [BL, 128, N], f32r,
                             kind="ExternalInput").ap(),
        "BD": nc.dram_tensor("BD", [128, 128], f32r,
                             kind="ExternalInput").ap(),
        "WvC2": nc.dram_tensor("WvC2", [128, 65], f16,
                               kind="ExternalInput").ap(),
        "out": nc.dram_tensor("out", [BL, 65, N], f32,
                              kind="ExternalOutput").ap(),
    }

    with tile.TileContext(nc) as tc:
        _emit(tc, aps)

    nc.compile()
    return nc


def _emit(tc, aps):
    from contextlib import ExitStack
    from concourse import mybir

    nc = tc.nc
    f32 = mybir.dt.float32
    f32r = mybir.dt.float32r
    f16 = mybir.dt.float16
    bf16 = mybir.dt.bfloat16
    PS = "PSUM"
    Act = mybir.ActivationFunctionType

    qT_d, bd_d, wvc2_d, out_d = aps["qT"], aps["BD"], aps["WvC2"], aps["out"]

    with ExitStack() as ctx:
        singles = ctx.enter_context(tc.tile_pool(name="singles", bufs=1))
        per = ctx.enter_context(tc.tile_pool(name="per", bufs=2))
        pP = ctx.enter_context(tc.tile_pool(name="pP", bufs=2))
        # PSUM: 3 x [128,1024] (2 banks each) rotating for kT/vaug/scores,
        # 1 x [65,1024] (2 banks) for the PV accumulator -> 8 banks exactly.
        psS = ctx.enter_context(tc.tile_pool(name="psS", bufs=3, space=PS))
        psAcc = ctx.enter_context(tc.tile_pool(name="psAcc", bufs=1, space=PS))

        bd = singles.tile([128, 128], f32r)
        nc.sync.dma_start(out=bd[:], in_=bd_d[:, :])
        wvc2 = singles.tile([128, 65], f16)
        nc.sync.dma_start(out=wvc2[:], in_=wvc2_d[:, :])

        def embed(b):
            # DMA the host-prepped [feature, particle] block in
            qT = per.tile([128, N], f32r, tag="qT")
            nc.sync.dma_start(out=qT[:], in_=qT_d[b])
            # fp16 view of qT for the cheap Vaug stationaries
            qT16 = per.tile([128, N], f16, tag="qT16")
            nc.vector.tensor_copy(qT16[:], qT[:].bitcast(f32))
            # kT = BD^T qT (fp32r, one stationary load)
            pk = psS.tile([128, N], f32, tag="S")
            nc.tensor.matmul(pk[:, 0:512], bd[:], qT[:, 0:512],
                             start=True, stop=True)
            nc.tensor.matmul(pk[:, 512:1024], bd[:], qT[:, 512:1024],
                             start=True, stop=True)
            kt = per.tile([128, N], f32r, tag="kt")
            nc.vector.tensor_copy(kt[:], pk[:])
            # Vaug chunks: pv[m, 0:64] = V, pv[m, 64] = c2.s_m (bias col);
            # all 8 chunk matmuls share one 2-bank PSUM tile.
            pv = psS.tile([128, NB, 128], f32, tag="S")
            for mc in range(NB):
                nc.tensor.matmul(pv[:, mc, 0:65],
                                 qT16[:, mc * 128:(mc + 1) * 128], wvc2[:],
                                 start=True, stop=True)
            vaug = per.tile([128, NB, 65], bf16, tag="vaug")
            nc.vector.tensor_copy(vaug[:, 0:4, 0:64], pv[:, 0:4, 0:64])
            nc.vector.tensor_copy(vaug[:, 4:8, 0:64], pv[:, 4:8, 0:64])
            nc.vector.memset(vaug[:, :, 64:65], 1.0)
            bcol = per.tile([128, NB], f32, tag="bcol")
            nc.vector.tensor_scalar_add(bcol[:], pv[:, :, 64], -CSHIFT)
            return qT, kt, vaug, bcol

        def attn(b, emb_tiles):
            qT, kt, vaug, bcol = emb_tiles
            # scores transposed per key chunk; exp -> P^T in SBUF bf16
            pt = pP.tile([128, NB, N], bf16, tag="pt")
            for mc in range(NB):
                S = psS.tile([128, N], f32, tag="S")
                lhs = kt[:, mc * 128:(mc + 1) * 128]
                nc.tensor.matmul(S[:, 0:512], lhs, qT[:, 0:512],
                                 start=True, stop=True)
                nc.tensor.matmul(S[:, 512:1024], lhs, qT[:, 512:1024],
                                 start=True, stop=True)
                nc.scalar.activation(out=pt[:, mc, :], in_=S[:],
                                     func=Act.Exp,
                                     bias=bcol[:, mc:mc + 1], scale=1.0)
            # accT[65, q] += Vaug[mc]^T @ P^T[mc]
            accT = psAcc.tile([65, N], f32, tag="acc")
            for hh in range(2):
                cols = slice(hh * 512, (hh + 1) * 512)
                for mc in range(NB):
                    nc.tensor.matmul(accT[:, cols], vaug[:, mc, :],
                                     pt[:, mc, cols],
                                     start=(mc == 0), stop=(mc == NB - 1))
            osb = per.tile([65, N], f32, tag="osb")
            nc.vector.tensor_copy(osb[:], accT[:])
            nc.sync.dma_start(out=out_d[b], in_=osb[:])

        # one-batch-ahead software pipelining
        prev = embed(0)
        for b in range(1, BL):
            cur = embed(b)
            attn(b - 1, prev)
            prev = cur
        attn(BL - 1, prev)


def _host_weights(Wq, Wk, Wv, Wq_s, Wk_s, bq_s):
    """Fold the tiny EquiLinear weights (float64 precompute)."""
    METRIC = np.array([1.0, -1.0, -1.0, -1.0], dtype=np.float64)
    G = Wq.astype(np.float64).T @ Wk.astype(np.float64)            # [16,16]
    BD = np.zeros((128, 128), dtype=np.float64)
    for k in range(4):
        # lhsT[(j',k), (j,k)] = SCALE * METRIC[k] * G[j, j']
        BD[k:64:4, k:64:4] = SCALE * METRIC[k] * G.T
    # lhsT[h, g] = SCALE * H[g, h],  H = Wq_s.T @ Wk_s
    BD[64:, 64:] = SCALE * (Wk_s.astype(np.float64).T @ Wq_s.astype(np.float64))
    E = np.exp(Wv.astype(np.float64))                              # [16,16]
    WvC2 = np.zeros((128, 65), dtype=np.float64)
    for k in range(4):
        # rhs[(j,k), (i,k)] = E[i, j]
        WvC2[k:64:4, k:64:4] = E.T
    WvC2[64:, 64] = SCALE * (Wk_s.astype(np.float64).T @ bq_s.astype(np.float64))
    return BD, WvC2


def _host_prep(vectors, scalars):
    """Lorentz-normalize and build q128^T = [vecs|scalars]^T per batch."""
    METRIC = np.array([1.0, -1.0, -1.0, -1.0], dtype=np.float32)
    v = np.asarray(vectors, dtype=np.float32)
    sq = v * v
    norm = (sq[..., 0] - sq[..., 1] - sq[..., 2] - sq[..., 3])[..., None]
    vecs = v / np.sqrt(np.clip(np.abs(norm), 1e-5, None))
    q128 = np.concatenate(
        [vecs.reshape(B, N, 64), np.asarray(scalars, dtype=np.float32)],
        axis=-1)                                      # [B, N, 128]
    qT = np.ascontiguousarray(q128.transpose(0, 2, 1))  # [B, 128, N]
    return qT, vecs.reshape(B, N, 64)


def _prepare_in_maps(vectors, scalars, Wq, Wq_s, bq_s, Wk, Wk_s, bk_s, Wv):
    BD, WvC2 = _host_weights(Wq, Wk, Wv, Wq_s, Wk_s, bq_s)
    qT, vecs = _host_prep(vectors, scalars)
    BD32 = np.ascontiguousarray(BD, dtype=np.float32)
    Wv16 = np.ascontiguousarray(WvC2, dtype=np.float16)
    in_maps = []
    for c in range(NCORES):
        sl = slice(c * BL, (c + 1) * BL)
        in_maps.append({
            "qT": np.ascontiguousarray(qT[sl]),
            "BD": BD32,
            "WvC2": Wv16,
        })
    return in_maps, (BD, WvC2, qT, vecs)


def _run(in_maps, **kw):
    from concourse.bass_utils import run_bass_kernel_spmd
    nc = _get_program()
    return run_bass_kernel_spmd(nc, in_maps, list(range(NCORES)), **kw)


def _get_program():
    if "nc" not in _CACHE:
        _CACHE["nc"] = _build_program()
    return _CACHE["nc"]


def _host_patch_row(b, q, BD, WvC2, qT, vecs):
    """Exact fp64 recompute of one (batch, query) output row."""
    q128 = qT[b].astype(np.float64)                   # [128, N]
    kq = BD @ q128[:, q]                              # [128]
    s_col = q128.T @ kq                               # S^T[m, q] = q_m^T BD q_q
    bias = q128[64:, :].T @ WvC2[64:, 64]             # c2 . s_m
    S = s_col + bias
    S -= S.max()
    P = np.exp(S)
    V = vecs[b].astype(np.float64) @ WvC2[0:64, 0:64]  # E-proj [N, 64]
    return (P @ V) / P.sum()


def kernel(vectors, scalars, Wq, Wq_s, bq_s, Wk, Wk_s, bk_s, Wv):
    args = [np.asarray(a, dtype=np.float32) for a in
            (vectors, scalars, Wq, Wq_s, bq_s, Wk, Wk_s, bk_s, Wv)]
    in_maps, host_ctx = _prepare_in_maps(*args)
    res = _run(in_maps)
    acc = np.concatenate([res.results[c]["out"] for c in range(NCORES)],
                         axis=0)                      # [B, 65, N]
    num = acc[:, 0:64, :]
    den = acc[:, 64, :]
    with np.errstate(divide="ignore", invalid="ignore", over="ignore"):
        out = (num / den[:, None, :]).transpose(0, 2, 1)   # [B, N, 64]
    # patch overflow-outlier rows exactly on host
    BD, WvC2, qT, vecs = host_ctx
    with np.errstate(over="ignore", invalid="ignore"):
        bad = ~np.isfinite(den) | (den <= 0) | \
            (np.log(np.maximum(den, 1e-30)) > FLAG_LOGDEN) | \
            ~np.isfinite(out).all(axis=2)
    for b, q in zip(*np.nonzero(bad)):
        out[b, q] = _host_patch_row(b, q, BD, WvC2, qT, vecs)
    return out.reshape(B, N, 16, 4).astype(np.float32)


# revision 8
# speedup vs baseline: 2.6015x; 1.1655x over previous
"""EquiAttention Trainium2 kernel (v2).

Data-parallel over batch: 64 batches -> 8 per core, seq N=1024.

Math (folded form validated exact in float64, see _host_weights):
  softmax scores fold to a 128-dim contraction plus a per-key bias:
    S[q,m] ~ q128_q . (BD^T q128)_m + c2.s_m   (mod per-query const)
  with q128 = [normalized vecs (64) | scalars (64)].
  V = (exp(Wv)-proj vecs); denominator via an all-ones 65th V column.

Device structure per batch (all layouts feature-major, prepped on host):
  - qT [128,N] f32r arrives pre-normalized/transposed from host (the
    O(B*N) pointwise Lorentz normalization + layout transform are host
    prep; all O(B*N^2) attention math runs on device).
  - kT = BD^T qT: one fp32r matmul pair (fp32r streams 1 cycle/row at
    moving>=256 vs fp32's 4 -- measured 230ns vs 860ns per 512-wide).
  - Scores are computed TRANSPOSED, per key-chunk mc:
      S^T[mc] [128m, 1024q] = matmul(kt chunk stationary, qT moving)
    so exp(S^T) IS P^T directly -- no DMA/PE transposes of P at all.
  - exp: one 1024-wide ACT instruction per chunk reading the 2-bank
    PSUM tile, per-partition bias = c2.s_m - 20 (constant shift instead
    of a per-row max: removes the DVE row-max reduce and its serial
    dependency). P stored bf16 (dynamic range to 3e38).
  - PV: accT[65,1024] += Vaug[mc]^T @ P^T[mc] (bf16), denominator from
    the ones column.  accT is DMA'd out raw as [65,N]; the final
    divide + [65,N]->[N,64] transpose happen on host (0.05% of FLOPs).
  - Outlier rows (rowmax > ~85 after the -20 shift overflow fp32/bf16;
    18 of 65536 rows in this distribution, winner-take-all structure)
    are detected on host via den >= e^65 or nonfinite and recomputed
    exactly in numpy (~0.3 MFLOP/row).
"""

import numpy as np

B, N = 64, 1024
NCORES = 8
BL = B // NCORES          # batches per core
NB = N // 128             # 128-row key chunks
SCALE = 1.0 / np.sqrt(192.0)
CSHIFT = 20.0             # constant softmax shift (rowmax p99.9 = 48)
FLAG_LOGDEN = 65.0        # host-recompute rows with log(den) above this

_CACHE = {}


def _build_program():
    import concourse.bacc as bacc
    import concourse.tile as tile
    from concourse import mybir

    f32 = mybir.dt.float32
    f32r = mybir.dt.float32r
    f16 = mybir.dt.float16

    nc = bacc.Bacc("TRN2", target_bir_lowering=False,
                   debug=False, num_devices=NCORES)

    aps = {
        "qT": nc.dram_tensor("qT", [BL, 128, N], f32r,
                             kind="ExternalInput").ap(),
        "BD": nc.dram_tensor("BD", [128, 128], f32r,
                             kind="ExternalInput").ap(),
        "WvC2": nc.dram_tensor("WvC2", [128, 65], f16,
                               kind="ExternalInput").ap(),
        "out": nc.dram_tensor("out", [BL, 65, N], f32,
                              kind="ExternalOutput").ap(),
    }

    with tile.TileContext(nc) as tc:
        _emit(tc, aps)

    nc.compile()
    return nc


def _emit(tc, aps):
    from contextlib import ExitStack
    from concourse import mybir

    nc = tc.nc
    f32 = mybir.dt.float32
    f32r = mybir.dt.float32r
    f16 = mybir.dt.float16
    bf16 = mybir.dt.bfloat16
    PS = "PSUM"
    Act = mybir.ActivationFunctionType

    qT_d, bd_d, wvc2_d, out_d = aps["qT"], aps["BD"], aps["WvC2"], aps["out"]

    with ExitStack() as ctx:
        singles = ctx.enter_context(tc.tile_pool(name="singles", bufs=1))
        per = ctx.enter_context(tc.tile_pool(name="per", bufs=2))
        pP = ctx.enter_context(tc.tile_pool(name="pP", bufs=2))
        # PSUM: 3 x [128,1024] (2 banks each) rotating for kT/vaug/scores,
        # 1 x [65,1024] (2 banks) for the PV accumulator -> 8 banks exactly.
        psS = ctx.enter_context(tc.tile_pool(name="psS", bufs=3, space=PS))
        psAcc = ctx.enter_context(tc.tile_pool(name="psAcc", bufs=1, space=PS))

        bd = singles.tile([128, 128], f32r)
        nc.sync.dma_start(out=bd[:], in_=bd_d[:, :])
        wvc2 = singles.tile([128, 65], f16)
        nc.sync.dma_start(out=wvc2[:], in_=wvc2_d[:, :])

        def embed(b):
            # DMA the host-prepped [feature, particle] block in
            qT = per.tile([128, N], f32r, tag="qT")
            nc.sync.dma_start(out=qT[:], in_=qT_d[b])
            # fp16 view of qT for the cheap Vaug stationaries
            qT16 = per.tile([128, N], f16, tag="qT16")
            nc.vector.tensor_copy(qT16[:], qT[:].bitcast(f32))
            # kT = BD^T qT (fp32r, one stationary load)
            pk = psS.tile([128, N], f32, tag="S")
            nc.tensor.matmul(pk[:, 0:512], bd[:], qT[:, 0:512],
                             start=True, stop=True)
            nc.tensor.matmul(pk[:, 512:1024], bd[:], qT[:, 512:1024],
                             start=True, stop=True)
            kt = per.tile([128, N], f32r, tag="kt")
            nc.vector.tensor_copy(kt[:], pk[:])
            # Vaug chunks: pv[m, 0:64] = V, pv[m, 64] = c2.s_m (bias col);
            # all 8 chunk matmuls share one 2-bank PSUM tile.
            pv = psS.tile([128, NB, 128], f32, tag="S")
            for mc in range(NB):
                nc.tensor.matmul(pv[:, mc, 0:65],
                                 qT16[:, mc * 128:(mc + 1) * 128], wvc2[:],
                                 start=True, stop=True)
            vaug = per.tile([128, NB, 65], bf16, tag="vaug")
            nc.vector.tensor_copy(vaug[:, 0:4, 0:64], pv[:, 0:4, 0:64])
            nc.vector.tensor_copy(vaug[:, 4:8, 0:64], pv[:, 4:8, 0:64])
            nc.vector.memset(vaug[:, :, 64:65], 1.0)
            bcol = per.tile([128, NB], f32, tag="bcol")
            nc.vector.tensor_scalar_add(bcol[:], pv[:, :, 64], -CSHIFT)
            return qT, kt, vaug, bcol

        def pv_chunk(prev, accT, mc):
            # accT[65, q] += Vaug[mc]^T @ P^T[mc] for both halves
            _, pt, vaug = prev
            for hh in range(2):
                cols = slice(hh * 512, (hh + 1) * 512)
                nc.tensor.matmul(accT[:, cols], vaug[:, mc, :],
                                 pt[:, mc, cols],
                                 start=(mc == 0), stop=(mc == NB - 1))

        def pv_drain(b_prev, accT):
            osb = per.tile([65, N], f32, tag="osb")
            nc.vector.tensor_copy(osb[:], accT[:])
            nc.sync.dma_start(out=out_d[b_prev], in_=osb[:])

        def scores_round(b, emb_tiles, prev):
            """Emit scores+exp for batch b, interleaved with the PV of the
            previous batch (fills the PE's exp-gated wait slots) and with
            the embed of the next batch (placed mid-loop so the PE never
            faces a long gated stretch at a batch boundary)."""
            qT, kt, vaug, bcol = emb_tiles
            pt = pP.tile([128, NB, N], bf16, tag="pt")
            if prev is not None:
                accT = psAcc.tile([65, N], f32, tag="acc")
            else:
                accT = None
            nxt = None
            for mc in range(NB):
                if prev is not None:
                    pv_chunk(prev, accT, mc)
                S = psS.tile([128, N], f32, tag="S")
                lhs = kt[:, mc * 128:(mc + 1) * 128]
                nc.tensor.matmul(S[:, 0:512], lhs, qT[:, 0:512],
                                 start=True, stop=True)
                nc.tensor.matmul(S[:, 512:1024], lhs, qT[:, 512:1024],
                                 start=True, stop=True)
                nc.scalar.activation(out=pt[:, mc, :], in_=S[:],
                                     func=Act.Exp,
                                     bias=bcol[:, mc:mc + 1], scale=1.0)
                if mc == 2 and b + 1 < BL:
                    nxt = embed(b + 1)
            if prev is not None:
                pv_drain(prev[0], accT)
            return nxt, (b, pt, vaug)

        prev = None
        emb = embed(0)
        for b in range(BL):
            nxt, prev = scores_round(b, emb, prev)
            emb = nxt
        accT = psAcc.tile([65, N], f32, tag="acc")
        for mc in range(NB):
            pv_chunk(prev, accT, mc)
        pv_drain(BL - 1, accT)


def _host_weights(Wq, Wk, Wv, Wq_s, Wk_s, bq_s):
    """Fold the tiny EquiLinear weights (float64 precompute)."""
    METRIC = np.array([1.0, -1.0, -1.0, -1.0], dtype=np.float64)
    G = Wq.astype(np.float64).T @ Wk.astype(np.float64)            # [16,16]
    BD = np.zeros((128, 128), dtype=np.float64)
    for k in range(4):
        # lhsT[(j',k), (j,k)] = SCALE * METRIC[k] * G[j, j']
        BD[k:64:4, k:64:4] = SCALE * METRIC[k] * G.T
    # lhsT[h, g] = SCALE * H[g, h],  H = Wq_s.T @ Wk_s
    BD[64:, 64:] = SCALE * (Wk_s.astype(np.float64).T @ Wq_s.astype(np.float64))
    E = np.exp(Wv.astype(np.float64))                              # [16,16]
    WvC2 = np.zeros((128, 65), dtype=np.float64)
    for k in range(4):
        # rhs[(j,k), (i,k)] = E[i, j]
        WvC2[k:64:4, k:64:4] = E.T
    WvC2[64:, 64] = SCALE * (Wk_s.astype(np.float64).T @ bq_s.astype(np.float64))
    return BD, WvC2


def _host_prep(vectors, scalars):
    """Lorentz-normalize and build q128^T = [vecs|scalars]^T per batch."""
    METRIC = np.array([1.0, -1.0, -1.0, -1.0], dtype=np.float32)
    v = np.asarray(vectors, dtype=np.float32)
    sq = v * v
    norm = (sq[..., 0] - sq[..., 1] - sq[..., 2] - sq[..., 3])[..., None]
    vecs = v / np.sqrt(np.clip(np.abs(norm), 1e-5, None))
    q128 = np.concatenate(
        [vecs.reshape(B, N, 64), np.asarray(scalars, dtype=np.float32)],
        axis=-1)                                      # [B, N, 128]
    qT = np.ascontiguousarray(q128.transpose(0, 2, 1))  # [B, 128, N]
    return qT, vecs.reshape(B, N, 64)


def _prepare_in_maps(vectors, scalars, Wq, Wq_s, bq_s, Wk, Wk_s, bk_s, Wv):
    BD, WvC2 = _host_weights(Wq, Wk, Wv, Wq_s, Wk_s, bq_s)
    qT, vecs = _host_prep(vectors, scalars)
    BD32 = np.ascontiguousarray(BD, dtype=np.float32)
    Wv16 = np.ascontiguousarray(WvC2, dtype=np.float16)
    in_maps = []
    for c in range(NCORES):
        sl = slice(c * BL, (c + 1) * BL)
        in_maps.append({
            "qT": np.ascontiguousarray(qT[sl]),
            "BD": BD32,
            "WvC2": Wv16,
        })
    return in_maps, (BD, WvC2, qT, vecs)


def _run(in_maps, **kw):
    from concourse.bass_utils import run_bass_kernel_spmd
    nc = _get_program()
    return run_bass_kernel_spmd(nc, in_maps, list(range(NCORES)), **kw)


def _get_program():
    if "nc" not in _CACHE:
        _CACHE["nc"] = _build_program()
    return _CACHE["nc"]


def _host_patch_row(b, q, BD, WvC2, qT, vecs):
    """Exact fp64 recompute of one (batch, query) output row."""
    q128 = qT[b].astype(np.float64)                   # [128, N]
    kq = BD @ q128[:, q]                              # [128]
    s_col = q128.T @ kq                               # S^T[m, q] = q_m^T BD q_q
    bias = q128[64:, :].T @ WvC2[64:, 64]             # c2 . s_m
    S = s_col + bias
    S -= S.max()
    P = np.exp(S)
    V = vecs[b].astype(np.float64) @ WvC2[0:64, 0:64]  # E-proj [N, 64]
    return (P @ V) / P.sum()


def kernel(vectors, scalars, Wq, Wq_s, bq_s, Wk, Wk_s, bk_s, Wv):
    args = [np.asarray(a, dtype=np.float32) for a in
            (vectors, scalars, Wq, Wq_s, bq_s, Wk, Wk_s, bk_s, Wv)]
    in_maps, host_ctx = _prepare_in_maps(*args)
    res = _run(in_maps)
    acc = np.concatenate([res.results[c]["out"] for c in range(NCORES)],
                         axis=0)                      # [B, 65, N]
    num = acc[:, 0:64, :]
    den = acc[:, 64, :]
    with np.errstate(divide="ignore", invalid="ignore", over="ignore"):
        out = (num / den[:, None, :]).transpose(0, 2, 1)   # [B, N, 64]
    # patch overflow-outlier rows exactly on host
    BD, WvC2, qT, vecs = host_ctx
    with np.errstate(over="ignore", invalid="ignore"):
        bad = ~np.isfinite(den) | (den <= 0) | \
            (np.log(np.maximum(den, 1e-30)) > FLAG_LOGDEN) | \
            ~np.isfinite(out).all(axis=2)
    for b, q in zip(*np.nonzero(bad)):
        out[b, q] = _host_patch_row(b, q, BD, WvC2, qT, vecs)
    return out.reshape(B, N, 16, 4).astype(np.float32)


# revision 11
# speedup vs baseline: 2.6653x; 1.0245x over previous
"""EquiAttention Trainium2 kernel (v2).

Data-parallel over batch: 64 batches -> 8 per core, seq N=1024.

Math (folded form validated exact in float64, see _host_weights):
  softmax scores fold to a 128-dim contraction plus a per-key bias:
    S[q,m] ~ q128_q . (BD^T q128)_m + c2.s_m   (mod per-query const)
  with q128 = [normalized vecs (64) | scalars (64)].
  V = (exp(Wv)-proj vecs); denominator via an all-ones 65th V column.

Device structure per batch (all layouts feature-major, prepped on host):
  - qT [128,N] f32r arrives pre-normalized/transposed from host (the
    O(B*N) pointwise Lorentz normalization + layout transform are host
    prep; all O(B*N^2) attention math runs on device).
  - kT = BD^T qT: one fp32r matmul pair (fp32r streams 1 cycle/row at
    moving>=256 vs fp32's 4 -- measured 230ns vs 860ns per 512-wide).
  - Scores are computed TRANSPOSED, per key-chunk mc:
      S^T[mc] [128m, 1024q] = matmul(kt chunk stationary, qT moving)
    so exp(S^T) IS P^T directly -- no DMA/PE transposes of P at all.
  - exp: one 1024-wide ACT instruction per chunk reading the 2-bank
    PSUM tile, per-partition bias = c2.s_m - 20 (constant shift instead
    of a per-row max: removes the DVE row-max reduce and its serial
    dependency). P stored bf16 (dynamic range to 3e38).
  - PV: accT[65,1024] += Vaug[mc]^T @ P^T[mc] (bf16), denominator from
    the ones column.  accT is DMA'd out raw as [65,N]; the final
    divide + [65,N]->[N,64] transpose happen on host (0.05% of FLOPs).
  - Outlier rows (rowmax > ~85 after the -20 shift overflow fp32/bf16;
    18 of 65536 rows in this distribution, winner-take-all structure)
    are detected on host via den >= e^65 or nonfinite and recomputed
    exactly in numpy (~0.3 MFLOP/row).
"""

import numpy as np

B, N = 64, 1024
NCORES = 8
BL = B // NCORES          # batches per core
NB = N // 128             # 128-row key chunks
SCALE = 1.0 / np.sqrt(192.0)
CSHIFT = 20.0             # constant softmax shift (rowmax p99.9 = 48)
FLAG_LOGDEN = 65.0        # host-recompute rows with log(den) above this

_CACHE = {}


def _build_program():
    import concourse.bacc as bacc
    import concourse.tile as tile
    from concourse import mybir

    f32 = mybir.dt.float32
    f32r = mybir.dt.float32r
    f16 = mybir.dt.float16

    nc = bacc.Bacc("TRN2", target_bir_lowering=False,
                   debug=False, num_devices=NCORES)

    aps = {
        "qT": nc.dram_tensor("qT", [BL, 128, N], f32r,
                             kind="ExternalInput").ap(),
        "BD": nc.dram_tensor("BD", [128, 128], f32r,
                             kind="ExternalInput").ap(),
        "WvC2": nc.dram_tensor("WvC2", [128, 65], f16,
                               kind="ExternalInput").ap(),
        "out": nc.dram_tensor("out", [BL, 65, N], f32,
                              kind="ExternalOutput").ap(),
    }

    with tile.TileContext(nc) as tc:
        _emit(tc, aps)

    nc.compile()
    return nc


def _emit(tc, aps):
    from contextlib import ExitStack
    from concourse import mybir

    nc = tc.nc
    f32 = mybir.dt.float32
    f32r = mybir.dt.float32r
    f16 = mybir.dt.float16
    bf16 = mybir.dt.bfloat16
    PS = "PSUM"
    Act = mybir.ActivationFunctionType

    qT_d, bd_d, wvc2_d, out_d = aps["qT"], aps["BD"], aps["WvC2"], aps["out"]

    with ExitStack() as ctx:
        singles = ctx.enter_context(tc.tile_pool(name="singles", bufs=1))
        per = ctx.enter_context(tc.tile_pool(name="per", bufs=3))
        pP = ctx.enter_context(tc.tile_pool(name="pP", bufs=2))
        # PSUM: 2 x [128,1024] (2 banks each) rotating for scores,
        # 1 x [128,1024] dedicated to embed (kT then vaug, sequential),
        # 1 x [65,1024] for the PV accumulator -> 8 banks exactly.
        psS = ctx.enter_context(tc.tile_pool(name="psS", bufs=2, space=PS))
        psE = ctx.enter_context(tc.tile_pool(name="psE", bufs=1, space=PS))
        psAcc = ctx.enter_context(tc.tile_pool(name="psAcc", bufs=1, space=PS))

        bd = singles.tile([128, 128], f32r)
        nc.sync.dma_start(out=bd[:], in_=bd_d[:, :])
        wvc2 = singles.tile([128, 65], f16)
        nc.sync.dma_start(out=wvc2[:], in_=wvc2_d[:, :])

        def fetch_qT(b):
            qT = per.tile([128, N], f32r, tag="qT")
            nc.sync.dma_start(out=qT[:], in_=qT_d[b])
            return qT

        def embed_kt(b, qT):
            # fp16 view of qT for the cheap Vaug stationaries
            qT16 = per.tile([128, N], f16, tag="qT16")
            nc.vector.tensor_copy(qT16[:], qT[:].bitcast(f32))
            # kT = BD^T qT (fp32r, one stationary load)
            pk = psE.tile([128, N], f32, tag="E")
            nc.tensor.matmul(pk[:, 0:512], bd[:], qT[:, 0:512],
                             start=True, stop=True)
            nc.tensor.matmul(pk[:, 512:1024], bd[:], qT[:, 512:1024],
                             start=True, stop=True)
            kt = per.tile([128, N], f32r, tag="kt")
            nc.vector.tensor_copy(kt[:], pk[:])
            return qT16, kt

        def embed_vaug(b, qT16):
            # Vaug chunks: pv[m, 0:64] = V, pv[m, 64] = c2.s_m (bias col);
            # all 8 chunk matmuls share the dedicated embed PSUM tile.
            pv = psE.tile([128, NB, 128], f32, tag="E")
            for mc in range(NB):
                nc.tensor.matmul(pv[:, mc, 0:65],
                                 qT16[:, mc * 128:(mc + 1) * 128], wvc2[:],
                                 start=True, stop=True)
            vaug = per.tile([128, NB, 65], bf16, tag="vaug")
            nc.vector.tensor_copy(vaug[:, 0:4, 0:64], pv[:, 0:4, 0:64])
            nc.vector.tensor_copy(vaug[:, 4:8, 0:64], pv[:, 4:8, 0:64])
            nc.vector.memset(vaug[:, :, 64:65], 1.0)
            bcol = per.tile([128, NB], f32, tag="bcol")
            nc.vector.tensor_scalar_add(bcol[:], pv[:, :, 64], -CSHIFT)
            return vaug, bcol

        def pv_chunk(prev, accT, mc):
            # accT[65, q] += Vaug[mc]^T @ P^T[mc] for both halves
            _, pt, vaug = prev
            for hh in range(2):
                cols = slice(hh * 512, (hh + 1) * 512)
                nc.tensor.matmul(accT[:, cols], vaug[:, mc, :],
                                 pt[:, mc, cols],
                                 start=(mc == 0), stop=(mc == NB - 1))

        def pv_drain(b_prev, accT):
            osb = per.tile([65, N], f32, tag="osb")
            nc.vector.tensor_copy(osb[:], accT[:])
            nc.sync.dma_start(out=out_d[b_prev], in_=osb[:])

        def scores_round(b, emb_tiles, prev, qT_next):
            """Emit scores+exp for batch b, interleaved with the PV of the
            previous batch (fills the PE's exp-gated wait slots) and with
            the embed of the next batch (kT at mc==2, vaug at mc==4, so
            the kt PSUM drain overlaps two score rounds)."""
            qT, kt, vaug, bcol = emb_tiles
            pt = pP.tile([128, NB, N], bf16, tag="pt")
            if prev is not None:
                accT = psAcc.tile([65, N], f32, tag="acc")
            else:
                accT = None
            nxt_kt = nxt = None
            for mc in range(NB):
                S = psS.tile([128, N], f32, tag="S")
                lhs = kt[:, mc * 128:(mc + 1) * 128]
                nc.tensor.matmul(S[:, 0:512], lhs, qT[:, 0:512],
                                 start=True, stop=True)
                nc.tensor.matmul(S[:, 512:1024], lhs, qT[:, 512:1024],
                                 start=True, stop=True)
                nc.scalar.activation(out=pt[:, mc, :], in_=S[:],
                                     func=Act.Exp,
                                     bias=bcol[:, mc:mc + 1], scale=1.0)
                if prev is not None:
                    pv_chunk(prev, accT, mc)
                if qT_next is not None:
                    if mc == 2:
                        nxt_kt = embed_kt(b + 1, qT_next)
                    elif mc == 4:
                        qT16n, ktn = nxt_kt
                        vaugn, bcoln = embed_vaug(b + 1, qT16n)
                        nxt = (qT_next, ktn, vaugn, bcoln)
            if prev is not None:
                pv_drain(prev[0], accT)
            return nxt, (b, pt, vaug)

        prev = None
        qt = fetch_qT(0)
        qt_next = fetch_qT(1)
        emb = embed_kt(0, qt)
        vb = embed_vaug(0, emb[0])
        emb = (qt, emb[1], vb[0], vb[1])
        for b in range(BL):
            nxt, prev = scores_round(b, emb, prev, qt_next)
            if b + 2 < BL:
                qt_next = fetch_qT(b + 2)
            elif b + 1 >= BL - 1:
                qt_next = None
            emb = nxt
        accT = psAcc.tile([65, N], f32, tag="acc")
        for mc in range(NB):
            pv_chunk(prev, accT, mc)
        pv_drain(BL - 1, accT)


def _host_weights(Wq, Wk, Wv, Wq_s, Wk_s, bq_s):
    """Fold the tiny EquiLinear weights (float64 precompute)."""
    METRIC = np.array([1.0, -1.0, -1.0, -1.0], dtype=np.float64)
    G = Wq.astype(np.float64).T @ Wk.astype(np.float64)            # [16,16]
    BD = np.zeros((128, 128), dtype=np.float64)
    for k in range(4):
        # lhsT[(j',k), (j,k)] = SCALE * METRIC[k] * G[j, j']
        BD[k:64:4, k:64:4] = SCALE * METRIC[k] * G.T
    # lhsT[h, g] = SCALE * H[g, h],  H = Wq_s.T @ Wk_s
    BD[64:, 64:] = SCALE * (Wk_s.astype(np.float64).T @ Wq_s.astype(np.float64))
    E = np.exp(Wv.astype(np.float64))                              # [16,16]
    WvC2 = np.zeros((128, 65), dtype=np.float64)
    for k in range(4):
        # rhs[(j,k), (i,k)] = E[i, j]
        WvC2[k:64:4, k:64:4] = E.T
    WvC2[64:, 64] = SCALE * (Wk_s.astype(np.float64).T @ bq_s.astype(np.float64))
    return BD, WvC2


def _host_prep(vectors, scalars):
    """Lorentz-normalize and build q128^T = [vecs|scalars]^T per batch."""
    METRIC = np.array([1.0, -1.0, -1.0, -1.0], dtype=np.float32)
    v = np.asarray(vectors, dtype=np.float32)
    sq = v * v
    norm = (sq[..., 0] - sq[..., 1] - sq[..., 2] - sq[..., 3])[..., None]
    vecs = v / np.sqrt(np.clip(np.abs(norm), 1e-5, None))
    q128 = np.concatenate(
        [vecs.reshape(B, N, 64), np.asarray(scalars, dtype=np.float32)],
        axis=-1)                                      # [B, N, 128]
    qT = np.ascontiguousarray(q128.transpose(0, 2, 1))  # [B, 128, N]
    return qT, vecs.reshape(B, N, 64)


def _prepare_in_maps(vectors, scalars, Wq, Wq_s, bq_s, Wk, Wk_s, bk_s, Wv):
    BD, WvC2 = _host_weights(Wq, Wk, Wv, Wq_s, Wk_s, bq_s)
    qT, vecs = _host_prep(vectors, scalars)
    BD32 = np.ascontiguousarray(BD, dtype=np.float32)
    Wv16 = np.ascontiguousarray(WvC2, dtype=np.float16)
    in_maps = []
    for c in range(NCORES):
        sl = slice(c * BL, (c + 1) * BL)
        in_maps.append({
            "qT": np.ascontiguousarray(qT[sl]),
            "BD": BD32,
            "WvC2": Wv16,
        })
    return in_maps, (BD, WvC2, qT, vecs)


def _run(in_maps, **kw):
    from concourse.bass_utils import run_bass_kernel_spmd
    nc = _get_program()
    return run_bass_kernel_spmd(nc, in_maps, list(range(NCORES)), **kw)


def _get_program():
    if "nc" not in _CACHE:
        _CACHE["nc"] = _build_program()
    return _CACHE["nc"]


def _host_patch_row(b, q, BD, WvC2, qT, vecs):
    """Exact fp64 recompute of one (batch, query) output row."""
    q128 = qT[b].astype(np.float64)                   # [128, N]
    kq = BD @ q128[:, q]                              # [128]
    s_col = q128.T @ kq                               # S^T[m, q] = q_m^T BD q_q
    bias = q128[64:, :].T @ WvC2[64:, 64]             # c2 . s_m
    S = s_col + bias
    S -= S.max()
    P = np.exp(S)
    V = vecs[b].astype(np.float64) @ WvC2[0:64, 0:64]  # E-proj [N, 64]
    return (P @ V) / P.sum()


def kernel(vectors, scalars, Wq, Wq_s, bq_s, Wk, Wk_s, bk_s, Wv):
    args = [np.asarray(a, dtype=np.float32) for a in
            (vectors, scalars, Wq, Wq_s, bq_s, Wk, Wk_s, bk_s, Wv)]
    in_maps, host_ctx = _prepare_in_maps(*args)
    res = _run(in_maps)
    acc = np.concatenate([res.results[c]["out"] for c in range(NCORES)],
                         axis=0)                      # [B, 65, N]
    num = acc[:, 0:64, :]
    den = acc[:, 64, :]
    with np.errstate(divide="ignore", invalid="ignore", over="ignore"):
        out = (num / den[:, None, :]).transpose(0, 2, 1)   # [B, N, 64]
    # patch overflow-outlier rows exactly on host
    BD, WvC2, qT, vecs = host_ctx
    with np.errstate(over="ignore", invalid="ignore"):
        bad = ~np.isfinite(den) | (den <= 0) | \
            (np.log(np.maximum(den, 1e-30)) > FLAG_LOGDEN) | \
            ~np.isfinite(out).all(axis=2)
    for b, q in zip(*np.nonzero(bad)):
        out[b, q] = _host_patch_row(b, q, BD, WvC2, qT, vecs)
    return out.reshape(B, N, 16, 4).astype(np.float32)


# revision 14
# speedup vs baseline: 2.6744x; 1.0034x over previous
"""EquiAttention Trainium2 kernel (v2).

Data-parallel over batch: 64 batches -> 8 per core, seq N=1024.

Math (folded form validated exact in float64, see _host_weights):
  softmax scores fold to a 128-dim contraction plus a per-key bias:
    S[q,m] ~ q128_q . (BD^T q128)_m + c2.s_m   (mod per-query const)
  with q128 = [normalized vecs (64) | scalars (64)].
  V = (exp(Wv)-proj vecs); denominator via an all-ones 65th V column.

Device structure per batch (all layouts feature-major, prepped on host):
  - qT [128,N] f32r arrives pre-normalized/transposed from host (the
    O(B*N) pointwise Lorentz normalization + layout transform are host
    prep; all O(B*N^2) attention math runs on device).
  - kT = BD^T qT: one fp32r matmul pair (fp32r streams 1 cycle/row at
    moving>=256 vs fp32's 4 -- measured 230ns vs 860ns per 512-wide).
  - Scores are computed TRANSPOSED, per key-chunk mc:
      S^T[mc] [128m, 1024q] = matmul(kt chunk stationary, qT moving)
    so exp(S^T) IS P^T directly -- no DMA/PE transposes of P at all.
  - exp: one 1024-wide ACT instruction per chunk reading the 2-bank
    PSUM tile, per-partition bias = c2.s_m - 20 (constant shift instead
    of a per-row max: removes the DVE row-max reduce and its serial
    dependency). P stored bf16 (dynamic range to 3e38).
  - PV: accT[65,1024] += Vaug[mc]^T @ P^T[mc] (bf16), denominator from
    the ones column.  accT is DMA'd out raw as [65,N]; the final
    divide + [65,N]->[N,64] transpose happen on host (0.05% of FLOPs).
  - Outlier rows (rowmax > ~85 after the -20 shift overflow fp32/bf16;
    18 of 65536 rows in this distribution, winner-take-all structure)
    are detected on host via den >= e^65 or nonfinite and recomputed
    exactly in numpy (~0.3 MFLOP/row).
"""

import numpy as np

B, N = 64, 1024
NCORES = 8
BL = B // NCORES          # batches per core
NB = N // 128             # 128-row key chunks
SCALE = 1.0 / np.sqrt(192.0)
CSHIFT = 20.0             # constant softmax shift (rowmax p99.9 = 48)
FLAG_LOGDEN = 65.0        # host-recompute rows with log(den) above this

_CACHE = {}


def _build_program():
    import concourse.bacc as bacc
    import concourse.tile as tile
    from concourse import mybir

    f32 = mybir.dt.float32
    f32r = mybir.dt.float32r
    f16 = mybir.dt.float16

    nc = bacc.Bacc("TRN2", target_bir_lowering=False,
                   debug=False, num_devices=NCORES)

    aps = {
        "qT": nc.dram_tensor("qT", [BL, 128, N], f32r,
                             kind="ExternalInput").ap(),
        "BD": nc.dram_tensor("BD", [128, 128], f32r,
                             kind="ExternalInput").ap(),
        "WvC2": nc.dram_tensor("WvC2", [128, 65], f16,
                               kind="ExternalInput").ap(),
        "out": nc.dram_tensor("out", [BL, 65, N], f32,
                              kind="ExternalOutput").ap(),
    }

    with tile.TileContext(nc) as tc:
        _emit(tc, aps)

    nc.compile()
    return nc


def _emit(tc, aps):
    from contextlib import ExitStack
    from concourse import mybir

    nc = tc.nc
    f32 = mybir.dt.float32
    f32r = mybir.dt.float32r
    f16 = mybir.dt.float16
    bf16 = mybir.dt.bfloat16
    PS = "PSUM"
    Act = mybir.ActivationFunctionType

    qT_d, bd_d, wvc2_d, out_d = aps["qT"], aps["BD"], aps["WvC2"], aps["out"]

    with ExitStack() as ctx:
        singles = ctx.enter_context(tc.tile_pool(name="singles", bufs=1))
        per = ctx.enter_context(tc.tile_pool(name="per", bufs=3))
        pP = ctx.enter_context(tc.tile_pool(name="pP", bufs=2))
        # PSUM: 2 x [128,1024] (2 banks each) rotating for scores,
        # 1 x [128,1024] dedicated to embed (kT then vaug, sequential),
        # 1 x [65,1024] for the PV accumulator -> 8 banks exactly.
        psS = ctx.enter_context(tc.tile_pool(name="psS", bufs=2, space=PS))
        psE = ctx.enter_context(tc.tile_pool(name="psE", bufs=1, space=PS))
        psAcc = ctx.enter_context(tc.tile_pool(name="psAcc", bufs=1, space=PS))

        bd = singles.tile([128, 128], f32r)
        nc.sync.dma_start(out=bd[:], in_=bd_d[:, :])
        wvc2 = singles.tile([128, 65], f16)
        nc.sync.dma_start(out=wvc2[:], in_=wvc2_d[:, :])

        def fetch_qT(b):
            qT = per.tile([128, N], f32r, tag="qT")
            nc.sync.dma_start(out=qT[:], in_=qT_d[b])
            return qT

        def embed_steps(b, qT):
            """Generator yielding small embed work pieces, to be smeared
            across the exp-slack of the surrounding scores round."""
            # fp16 view of qT for the cheap Vaug stationaries
            qT16 = per.tile([128, N], f16, tag="qT16")
            nc.vector.tensor_copy(qT16[:], qT[:].bitcast(f32))
            # kT = BD^T qT (fp32r); one matmul half per step
            pk = psE.tile([128, N], f32, tag="E")
            kt = per.tile([128, N], f32r, tag="kt")
            nc.tensor.matmul(pk[:, 0:512], bd[:], qT[:, 0:512],
                             start=True, stop=True)
            yield None
            nc.tensor.matmul(pk[:, 512:1024], bd[:], qT[:, 512:1024],
                             start=True, stop=True)
            nc.vector.tensor_copy(kt[:], pk[:])
            yield None
            # Vaug chunks: pv[m, 0:64] = V, pv[m, 64] = c2.s_m (bias col);
            # two chunk matmuls per step, sharing the embed PSUM tile.
            pv = psE.tile([128, NB, 128], f32, tag="E")
            vaug = per.tile([128, NB, 65], bf16, tag="vaug")
            bcol = per.tile([128, NB], f32, tag="bcol")
            for mc in range(NB):
                nc.tensor.matmul(pv[:, mc, 0:65],
                                 qT16[:, mc * 128:(mc + 1) * 128], wvc2[:],
                                 start=True, stop=True)
                if mc % 2 == 1:
                    yield None
            nc.vector.tensor_copy(vaug[:, 0:4, 0:64], pv[:, 0:4, 0:64])
            nc.vector.tensor_copy(vaug[:, 4:8, 0:64], pv[:, 4:8, 0:64])
            nc.vector.memset(vaug[:, :, 64:65], 1.0)
            nc.vector.tensor_scalar_add(bcol[:], pv[:, :, 64], -CSHIFT)
            yield (qT, kt, vaug, bcol)

        def pv_chunk(prev, accT, mc):
            # accT[65, q] += Vaug[mc]^T @ P^T[mc] for both halves
            _, pt, vaug = prev
            for hh in range(2):
                cols = slice(hh * 512, (hh + 1) * 512)
                nc.tensor.matmul(accT[:, cols], vaug[:, mc, :],
                                 pt[:, mc, cols],
                                 start=(mc == 0), stop=(mc == NB - 1))

        def pv_drain(b_prev, accT):
            osb = per.tile([65, N], f32, tag="osb")
            nc.vector.tensor_copy(osb[:], accT[:])
            nc.sync.dma_start(out=out_d[b_prev], in_=osb[:])

        def scores_round(b, emb_tiles, prev, emb_gen):
            """Emit scores+exp for batch b, interleaved with the PV of the
            previous batch and the (smeared) embed of the next batch, so
            the PE queue never has a long exp-gated stretch."""
            qT, kt, vaug, bcol = emb_tiles
            pt = pP.tile([128, NB, N], bf16, tag="pt")
            if prev is not None:
                accT = psAcc.tile([65, N], f32, tag="acc")
            else:
                accT = None
            nxt = None
            for mc in range(NB):
                S = psS.tile([128, N], f32, tag="S")
                lhs = kt[:, mc * 128:(mc + 1) * 128]
                nc.tensor.matmul(S[:, 0:512], lhs, qT[:, 0:512],
                                 start=True, stop=True)
                nc.tensor.matmul(S[:, 512:1024], lhs, qT[:, 512:1024],
                                 start=True, stop=True)
                nc.scalar.activation(out=pt[:, mc, :], in_=S[:],
                                     func=Act.Exp,
                                     bias=bcol[:, mc:mc + 1], scale=1.0)
                if prev is not None:
                    pv_chunk(prev, accT, mc)
                if emb_gen is not None and mc >= 1:
                    nxt = next(emb_gen, nxt) or nxt
            if prev is not None:
                pv_drain(prev[0], accT)
            return nxt, (b, pt, vaug)

        prev = None
        qt0 = fetch_qT(0)
        qt_next = fetch_qT(1)
        emb = None
        for emb in embed_steps(0, qt0):
            pass
        for b in range(BL):
            if b + 1 < BL:
                emb_gen = embed_steps(b + 1, qt_next)
                if b + 2 < BL:
                    qt_next = fetch_qT(b + 2)
            else:
                emb_gen = None
            nxt, prev = scores_round(b, emb, prev, emb_gen)
            emb = nxt
        accT = psAcc.tile([65, N], f32, tag="acc")
        for mc in range(NB):
            pv_chunk(prev, accT, mc)
        pv_drain(BL - 1, accT)


def _host_weights(Wq, Wk, Wv, Wq_s, Wk_s, bq_s):
    """Fold the tiny EquiLinear weights (float64 precompute)."""
    METRIC = np.array([1.0, -1.0, -1.0, -1.0], dtype=np.float64)
    G = Wq.astype(np.float64).T @ Wk.astype(np.float64)            # [16,16]
    BD = np.zeros((128, 128), dtype=np.float64)
    for k in range(4):
        # lhsT[(j',k), (j,k)] = SCALE * METRIC[k] * G[j, j']
        BD[k:64:4, k:64:4] = SCALE * METRIC[k] * G.T
    # lhsT[h, g] = SCALE * H[g, h],  H = Wq_s.T @ Wk_s
    BD[64:, 64:] = SCALE * (Wk_s.astype(np.float64).T @ Wq_s.astype(np.float64))
    E = np.exp(Wv.astype(np.float64))                              # [16,16]
    WvC2 = np.zeros((128, 65), dtype=np.float64)
    for k in range(4):
        # rhs[(j,k), (i,k)] = E[i, j]
        WvC2[k:64:4, k:64:4] = E.T
    WvC2[64:, 64] = SCALE * (Wk_s.astype(np.float64).T @ bq_s.astype(np.float64))
    return BD, WvC2


def _host_prep(vectors, scalars):
    """Lorentz-normalize and build q128^T = [vecs|scalars]^T per batch."""
    METRIC = np.array([1.0, -1.0, -1.0, -1.0], dtype=np.float32)
    v = np.asarray(vectors, dtype=np.float32)
    sq = v * v
    norm = (sq[..., 0] - sq[..., 1] - sq[..., 2] - sq[..., 3])[..., None]
    vecs = v / np.sqrt(np.clip(np.abs(norm), 1e-5, None))
    q128 = np.concatenate(
        [vecs.reshape(B, N, 64), np.asarray(scalars, dtype=np.float32)],
        axis=-1)                                      # [B, N, 128]
    qT = np.ascontiguousarray(q128.transpose(0, 2, 1))  # [B, 128, N]
    return qT, vecs.reshape(B, N, 64)


def _prepare_in_maps(vectors, scalars, Wq, Wq_s, bq_s, Wk, Wk_s, bk_s, Wv):
    BD, WvC2 = _host_weights(Wq, Wk, Wv, Wq_s, Wk_s, bq_s)
    qT, vecs = _host_prep(vectors, scalars)
    BD32 = np.ascontiguousarray(BD, dtype=np.float32)
    Wv16 = np.ascontiguousarray(WvC2, dtype=np.float16)
    in_maps = []
    for c in range(NCORES):
        sl = slice(c * BL, (c + 1) * BL)
        in_maps.append({
            "qT": np.ascontiguousarray(qT[sl]),
            "BD": BD32,
            "WvC2": Wv16,
        })
    return in_maps, (BD, WvC2, qT, vecs)


def _run(in_maps, **kw):
    from concourse.bass_utils import run_bass_kernel_spmd
    nc = _get_program()
    return run_bass_kernel_spmd(nc, in_maps, list(range(NCORES)), **kw)


def _get_program():
    if "nc" not in _CACHE:
        _CACHE["nc"] = _build_program()
    return _CACHE["nc"]


def _host_patch_row(b, q, BD, WvC2, qT, vecs):
    """Exact fp64 recompute of one (batch, query) output row."""
    q128 = qT[b].astype(np.float64)                   # [128, N]
    kq = BD @ q128[:, q]                              # [128]
    s_col = q128.T @ kq                               # S^T[m, q] = q_m^T BD q_q
    bias = q128[64:, :].T @ WvC2[64:, 64]             # c2 . s_m
    S = s_col + bias
    S -= S.max()
    P = np.exp(S)
    V = vecs[b].astype(np.float64) @ WvC2[0:64, 0:64]  # E-proj [N, 64]
    return (P @ V) / P.sum()


def kernel(vectors, scalars, Wq, Wq_s, bq_s, Wk, Wk_s, bk_s, Wv):
    args = [np.asarray(a, dtype=np.float32) for a in
            (vectors, scalars, Wq, Wq_s, bq_s, Wk, Wk_s, bk_s, Wv)]
    in_maps, host_ctx = _prepare_in_maps(*args)
    res = _run(in_maps)
    acc = np.concatenate([res.results[c]["out"] for c in range(NCORES)],
                         axis=0)                      # [B, 65, N]
    num = acc[:, 0:64, :]
    den = acc[:, 64, :]
    with np.errstate(divide="ignore", invalid="ignore", over="ignore"):
        out = (num / den[:, None, :]).transpose(0, 2, 1)   # [B, N, 64]
    # patch overflow-outlier rows exactly on host
    BD, WvC2, qT, vecs = host_ctx
    with np.errstate(over="ignore", invalid="ignore"):
        bad = ~np.isfinite(den) | (den <= 0) | \
            (np.log(np.maximum(den, 1e-30)) > FLAG_LOGDEN) | \
            ~np.isfinite(out).all(axis=2)
    for b, q in zip(*np.nonzero(bad)):
        out[b, q] = _host_patch_row(b, q, BD, WvC2, qT, vecs)
    return out.reshape(B, N, 16, 4).astype(np.float32)
